# revision 5
# baseline (speedup 1.0000x reference)
import os
import sys

sys.path.insert(0, '/opt/trn_rl_repo')

import numpy as np

import concourse.bass as bass
import concourse.mybir as mybir
import concourse.tile as tile
from concourse import bacc
from concourse.bass_utils import run_bass_kernel_spmd

# ---- model dims (hardcoded from the problem spec) ----
B, L, D = 2, 1024, 2048
H = 16
NOPE, ROPE_D, VH = 128, 64, 128
QKD = NOPE + ROPE_D
Q_LORA, KV_LORA = 1024, 512
FFN, EFFN = 8192, 1024
NE, NZ, TOPK = 8, 2, 2
RSF = 1.5
EPS = 1e-5
ROPE_BASE = 10000.0
SCALE = QKD ** -0.5
SQ = (D / Q_LORA) ** 0.5
SKV = (D / KV_LORA) ** 0.5

N_CORES = 8
T = 256            # tokens per core
NC_D = D // 128    # 16 feature chunks
NKT = L // 128     # 8 key tiles per batch
MASK_NEG = -10240.0

F32 = mybir.dt.float32
F32R = mybir.dt.float32r
F16 = mybir.dt.float16
AF = mybir.ActivationFunctionType
ALU = mybir.AluOpType
AX = mybir.AxisListType

DEBUG = bool(int(os.environ.get("KERNEL_DEBUG", "0")))


# ---------------- host-side packing helpers ----------------

def _pack_lhsT(wT, np_dtype):
    """wT [K, M] (K=KC*128, M padded to MB*128) -> [MB, 128, KC*128]
    where pack[mb, p, kc*128+m] = wT[kc*128+p, mb*128+m]."""
    K, M = wT.shape
    KC = K // 128
    MB = (M + 127) // 128
    if M != MB * 128:
        wT = np.concatenate([wT, np.zeros((K, MB * 128 - M), wT.dtype)], axis=1)
    a = wT.reshape(KC, 128, MB, 128)
    a = a.transpose(2, 1, 0, 3).reshape(MB, 128, KC * 128)
    return np.ascontiguousarray(a.astype(np_dtype))


def _pack_rhs(wT, np_dtype):
    """wT [K, F] -> [128, KC*F]: pack[p, kc*F+f] = wT[kc*128+p, f]."""
    K, F = wT.shape
    KC = K // 128
    a = wT.reshape(KC, 128, F).transpose(1, 0, 2).reshape(128, KC * F)
    return np.ascontiguousarray(a.astype(np_dtype))


def _prep_inputs(inp):
    f32 = np.float32
    in_maps = []
    common = {}

    mask = np.asarray(inp['mask'])
    x = np.asarray(inp['x'], dtype=f32)

    for i in range(2):
        in_ln = np.asarray(inp[f'in_ln{i}'], f32)
        post_ln = np.asarray(inp[f'post_ln{i}'], f32)
        qa_w = np.asarray(inp[f'a{i}_qa_w'], f32)
        qa_ln = np.asarray(inp[f'a{i}_qa_ln'], f32)
        qb_w = np.asarray(inp[f'a{i}_qb_w'], f32)
        kva_w = np.asarray(inp[f'a{i}_kva_w'], f32)
        kva_ln = np.asarray(inp[f'a{i}_kva_ln'], f32)
        embq = np.asarray(inp[f'a{i}_embq'], f32)
        unemb = np.asarray(inp[f'a{i}_unemb'], f32)
        o_w = np.asarray(inp[f'a{i}_o_w'], f32)

        common[f'qa{i}'] = _pack_lhsT((qa_w * in_ln[None, :]).T, f32)
        qb_eff_T = (qb_w * qa_ln[None, :]).T * (SQ * SCALE)   # [Q_LORA, H*QKD]
        common[f'qbn{i}'] = np.stack(
            [_pack_lhsT(qb_eff_T[:, h * QKD:h * QKD + NOPE], f32)[0]
             for h in range(H)])
        common[f'qbp{i}'] = np.stack([
            _pack_lhsT(np.concatenate([
                qb_eff_T[:, (2 * hp) * QKD + NOPE:(2 * hp) * QKD + QKD],
                qb_eff_T[:, (2 * hp + 1) * QKD + NOPE:(2 * hp + 1) * QKD + QKD],
            ], axis=1), f32)[0] for hp in range(H // 2)])
        common[f'kva{i}'] = _pack_lhsT((kva_w * in_ln[None, :]).T, f32)
        emb_scale = (kva_ln * SKV)
        common[f'embq{i}'] = np.stack([
            _pack_lhsT(embq[h] * emb_scale[:, None], f32)[0] for h in range(H)])
        common[f'unemb{i}'] = np.stack([
            _pack_rhs(np.concatenate([
                (unemb[2 * hp] * emb_scale[None, :]).T,
                (unemb[2 * hp + 1] * emb_scale[None, :]).T], axis=1), f32)
            for hp in range(H // 2)])
        common[f'ow{i}'] = _pack_lhsT(o_w.T, f32)

        gate = np.asarray(inp[f'm{i}_gate'], f32)
        up = np.asarray(inp[f'm{i}_up'], f32)
        down = np.asarray(inp[f'm{i}_down'], f32)
        common[f'gate{i}'] = _pack_lhsT((gate * post_ln[None, :]).T, np.float16)
        common[f'up{i}'] = _pack_lhsT((up * post_ln[None, :]).T, np.float16)
        common[f'down{i}'] = _pack_lhsT(down.T, np.float16)

    post_ln0 = np.asarray(inp['post_ln0'], f32)
    router_w = np.asarray(inp['router_w'], f32)
    e_gate = np.asarray(inp['e_gate'], f32)
    e_up = np.asarray(inp['e_up'], f32)
    e_down = np.asarray(inp['e_down'], f32)
    common['router'] = _pack_rhs((router_w * post_ln0[None, :]).T, f32)
    common['ebias'] = np.asarray(inp['e_bias'], f32).reshape(1, NE + NZ)
    common['egate'] = np.stack([
        _pack_lhsT((e_gate[e] * post_ln0[None, :]).T, np.float16)
        for e in range(NE)])
    common['eup'] = np.stack([
        _pack_lhsT((e_up[e] * post_ln0[None, :]).T, np.float16)
        for e in range(NE)])
    common['edown'] = np.stack([
        _pack_lhsT(e_down[e].T, np.float16) for e in range(NE)])
    common['postln0'] = np.ascontiguousarray(post_ln0.reshape(NC_D, 128).T)
    common['ones_r'] = np.ones((128, 128), f32)

    half = ROPE_D // 2
    inv = ROPE_BASE ** (-np.arange(half, dtype=f32) * 2.0 / ROPE_D)

    for c in range(N_CORES):
        b, blk = c // 4, c % 4
        m = dict(common)
        xs = x[b, blk * T:(blk + 1) * T, :]                     # [256, 2048]
        m['xt'] = np.ascontiguousarray(
            xs.T.reshape(NC_D, 128, T).transpose(1, 0, 2).reshape(128, NC_D * T))
        mb = np.empty((NKT, 128, T), np.float16)
        for kt in range(NKT):
            sub = mask[blk * T:(blk + 1) * T, kt * 128:(kt + 1) * 128]   # [q, k]
            mb[kt] = np.where(sub.T, 0.0, MASK_NEG).astype(np.float16)
        m['masks'] = mb
        pos = np.arange(blk * T, (blk + 1) * T, dtype=f32)
        ang = inv[:, None] * pos[None, :]                        # [32, 256]
        c64 = np.repeat(np.cos(ang), 2, axis=0)                  # [64, 256]
        s64 = np.repeat(np.sin(ang), 2, axis=0)
        sgn = np.where(np.arange(64) % 2 == 0, -1.0, 1.0).astype(f32)
        s64 = s64 * sgn[:, None]
        m['ropec'] = np.ascontiguousarray(np.concatenate([c64, c64], axis=0))
        m['ropes'] = np.ascontiguousarray(np.concatenate([s64, s64], axis=0))
        in_maps.append(m)
    return in_maps


# ---------------- device program ----------------

def _build_program():
    nc = bacc.Bacc("TRN2", target_bir_lowering=False, debug=False,
                   num_devices=N_CORES)

    def din(name, shape, dt):
        return nc.dram_tensor(name, list(shape), dt, kind="ExternalInput").ap()

    inn = {}
    for i in range(2):
        inn[f'qa{i}'] = din(f'qa{i}', [8, 128, 2048], F32R)
        inn[f'qbn{i}'] = din(f'qbn{i}', [H, 128, 1024], F32R)
        inn[f'qbp{i}'] = din(f'qbp{i}', [H // 2, 128, 1024], F32R)
        inn[f'kva{i}'] = din(f'kva{i}', [5, 128, 2048], F32R)
        inn[f'embq{i}'] = din(f'embq{i}', [H, 128, 512], F32R)
        inn[f'unemb{i}'] = din(f'unemb{i}', [H // 2, 128, 1024], F32R)
        inn[f'ow{i}'] = din(f'ow{i}', [16, 128, 2048], F32R)
        inn[f'gate{i}'] = din(f'gate{i}', [64, 128, 2048], F16)
        inn[f'up{i}'] = din(f'up{i}', [64, 128, 2048], F16)
        inn[f'down{i}'] = din(f'down{i}', [16, 128, 8192], F16)
    inn['router'] = din('router', [128, 160], F32)
    inn['ebias'] = din('ebias', [1, NE + NZ], F32)
    inn['egate'] = din('egate', [NE, 8, 128, 2048], F16)
    inn['eup'] = din('eup', [NE, 8, 128, 2048], F16)
    inn['edown'] = din('edown', [NE, 16, 128, 1024], F16)
    inn['postln0'] = din('postln0', [128, NC_D], F32)
    inn['ones_r'] = din('ones_r', [128, 128], F32R)
    inn['xt'] = din('xt', [128, NC_D * T], F32)
    inn['masks'] = din('masks', [NKT, 128, T], F16)
    inn['ropec'] = din('ropec', [128, T], F32)
    inn['ropes'] = din('ropes', [128, T], F32)

    out_d = nc.dram_tensor("out", [128, NC_D * T], F32, kind="ExternalOutput").ap()
    dbg = {}
    if DEBUG:
        for name in ["dbg_h0", "dbg_hn0", "dbg_short", "dbg_h1"]:
            dbg[name] = nc.dram_tensor(name, [128, NC_D * T], F32,
                                       kind="ExternalOutput").ap()
        dbg["dbg_ce"] = nc.dram_tensor("dbg_ce", [2, 128, NE + NZ], F32,
                                       kind="ExternalOutput").ap()

    ag_in = [nc.dram_tensor(f"ag_in{i}", [576, T], F32R).ap() for i in range(2)]
    ag_out = [nc.dram_tensor(f"ag_out{i}", [4 * 576, T], F32R).ap()
              for i in range(2)]
    groups = [[0, 1, 2, 3], [4, 5, 6, 7]]

    with tile.TileContext(nc) as tc:
        _emit(tc, inn, out_d, dbg, ag_in, ag_out, groups)

    nc.compile()
    return nc


def _rms_norm(tc, src, n_chunks, d_red, outs, scr, ps, ones_r, epst):
    """outs: list of tiles shaped [128, n_chunks, T]; out = src * rstd."""
    nc = tc.nc
    ms = ps.tile([128, T], F32, tag="mm0_ps", name="rms_ms")
    for c in range(n_chunks):
        xsq = scr.tile([128, T], F32R, tag="xsq", name="rms_xsq")
        nc.scalar.activation(xsq[:], src[:, c, :], AF.Square)
        nc.tensor.matmul(ms[:], ones_r[:], xsq[:],
                         start=(c == 0), stop=(c == n_chunks - 1))
    sd = scr.tile([128, T], F32, tag="rms_sd", name="rms_sd")
    nc.scalar.activation(sd[:], ms[:], AF.Sqrt, scale=1.0 / d_red, bias=epst[:])
    rstd = scr.tile([128, T], F32, tag="rms_rstd", name="rms_rstd")
    nc.vector.reciprocal(rstd[:], sd[:])
    for c in range(n_chunks):
        for ot in outs:
            nc.vector.tensor_tensor(ot[:, c, :], src[:, c, :], rstd[:], ALU.mult)


def _rope(tc, scr, out_ap, src_ps, npart, ropec, ropes):
    """out = src*cos + pairswap(src)*sin_signed; src_ps is a PSUM AP."""
    nc = tc.nc
    raw = scr.tile([128, T], F32, tag="rope_raw", name="rope_raw")
    nc.vector.tensor_copy(raw[0:npart, :], src_ps)
    sw = scr.tile([128, T], F32, tag="rope_sw", name="rope_sw")
    nc.sync.dma_start(sw[0:npart:2, :], raw[1:npart:2, :])
    nc.sync.dma_start(sw[1:npart:2, :], raw[0:npart:2, :])
    t1 = scr.tile([128, T], F32, tag="rope_t1", name="rope_t1")
    nc.vector.tensor_tensor(t1[0:npart, :], raw[0:npart, :],
                            ropec[0:npart, :], ALU.mult)
    t2 = scr.tile([128, T], F32, tag="rope_t2", name="rope_t2")
    nc.vector.tensor_tensor(t2[0:npart, :], sw[0:npart, :],
                            ropes[0:npart, :], ALU.mult)
    nc.vector.tensor_tensor(out_ap, t1[0:npart, :], t2[0:npart, :], ALU.add)


def _emit(tc, inn, out_d, dbg, ag_in, ag_out, groups):
    nc = tc.nc
    with tc.tile_pool(name="persist", bufs=1) as per:
        h_t = per.tile([128, NC_D, T], F32, name="h_t")
        short_t = per.tile([128, NC_D, T], F32, name="short_t")
        ones_r = per.tile([128, 128], F32R, name="ones_r_sb")
        nc.sync.dma_start(ones_r[:], inn['ones_r'][:])
        epst = per.tile([128, 1], F32, name="epst")
        nc.vector.memset(epst[:], EPS)
        ropec = per.tile([128, T], F32, name="ropec_sb")
        ropes = per.tile([128, T], F32, name="ropes_sb")
        nc.sync.dma_start(ropec[:], inn['ropec'][:])
        nc.sync.dma_start(ropes[:], inn['ropes'][:])
        masks = per.tile([128, NKT, T], F16, name="masks_sb")
        for kt in range(NKT):
            nc.sync.dma_start(masks[:, kt, :], inn['masks'][kt])
        postln0 = per.tile([128, NC_D], F32, name="postln0_sb")
        nc.sync.dma_start(postln0[:], inn['postln0'][:])
        ebias = per.tile([128, NE + NZ], F32, name="ebias_sb")
        nc.sync.dma_start(ebias[:], bass.AP(
            tensor=inn['ebias'].tensor, offset=inn['ebias'].offset,
            ap=[[0, 128], [1, NE + NZ]]))
        nc.sync.dma_start(h_t[:], inn['xt'][:])

        for i in range(2):
            _emit_attn(tc, i, inn, h_t, ones_r, epst, ropec, ropes, masks,
                       ag_in, ag_out, groups)
            if DEBUG and i == 0:
                for c in range(NC_D):
                    nc.sync.dma_start(dbg["dbg_h0"][:, c * T:(c + 1) * T],
                                      h_t[:, c, :])
            _emit_ffn(tc, i, inn, dbg, h_t, short_t, ones_r, epst, postln0,
                      ebias)
            if DEBUG and i == 0:
                for c in range(NC_D):
                    nc.sync.dma_start(dbg["dbg_h1"][:, c * T:(c + 1) * T],
                                      h_t[:, c, :])

        with tc.tile_pool(name="fin", bufs=3) as fin:
            for c in range(NC_D):
                o = fin.tile([128, T], F32, tag="fin_o", name="fin_o")
                nc.vector.tensor_tensor(o[:], h_t[:, c, :], short_t[:, c, :],
                                        ALU.add)
                nc.sync.dma_start(out_d[:, c * T:(c + 1) * T], o[:])


def _emit_attn(tc, i, inn, h_t, ones_r, epst, ropec, ropes, masks,
               ag_in, ag_out, groups):
    nc = tc.nc
    with (
        tc.tile_pool(name=f"a{i}_act", bufs=1) as act,
        tc.tile_pool(name=f"a{i}_scr", bufs=2) as scr,
        tc.tile_pool(name=f"a{i}_w", bufs=2) as wp,
        tc.tile_pool(name=f"a{i}_ps", bufs=2, space="PSUM") as ps,
    ):
        qnope = act.tile([128, H, T], F32R, name="qnope")
        qpe = act.tile([128, H // 2, T], F32R, name="qpe")
        kvn = act.tile([128, 4, T], F32R, name="kvn")
        kpe = act.tile([64, T], F32R, name="kpe")
        kvk = act.tile([128, 4, L], F32R, name="kvk")
        kpek = act.tile([128, L], F32R, name="kpek")
        attn = act.tile([128, H, T], F32R, name="attn")

        with tc.tile_pool(name=f"a{i}_pre", bufs=1) as pre:
            rn = pre.tile([128, NC_D, T], F32R, name="rn")
            _rms_norm(tc, h_t, NC_D, D, [rn], scr, ps, ones_r, epst)

            cq = pre.tile([128, 8, T], F32, name="cq")
            for mb in range(8):
                w = wp.tile([128, NC_D, 128], F32R, tag="big_w", name="qa_w")
                nc.sync.dma_start(w[:], inn[f'qa{i}'][mb])
                p = ps.tile([128, T], F32, tag="mm0_ps", name="qa_ps")
                for kc in range(NC_D):
                    nc.tensor.matmul(p[:], w[:, kc, :], rn[:, kc, :],
                                     start=(kc == 0), stop=(kc == NC_D - 1))
                nc.vector.tensor_copy(cq[:, mb, :], p[:])
            cqn = pre.tile([128, 8, T], F32R, name="cqn")
            _rms_norm(tc, cq, 8, Q_LORA, [cqn], scr, ps, ones_r, epst)

            for h in range(H):
                w = wp.tile([128, 8, 128], F32R, tag="qb_w", name="qbn_w")
                nc.sync.dma_start(w[:], inn[f'qbn{i}'][h])
                p = ps.tile([128, T], F32, tag="mm0_ps", name="qbn_ps")
                for kc in range(8):
                    nc.tensor.matmul(p[:], w[:, kc, :], cqn[:, kc, :],
                                     start=(kc == 0), stop=(kc == 7))
                nc.vector.tensor_copy(qnope[:, h, :], p[:])
            for hp in range(H // 2):
                w = wp.tile([128, 8, 128], F32R, tag="qb_w", name="qbp_w")
                nc.sync.dma_start(w[:], inn[f'qbp{i}'][hp])
                p = ps.tile([128, T], F32, tag="mm0_ps", name="qbp_ps")
                for kc in range(8):
                    nc.tensor.matmul(p[:], w[:, kc, :], cqn[:, kc, :],
                                     start=(kc == 0), stop=(kc == 7))
                _rope(tc, scr, qpe[:, hp, :], p[:], 128, ropec, ropes)

            kvraw = pre.tile([128, 4, T], F32, name="kvraw")
            for mb in range(5):
                w = wp.tile([128, NC_D, 128], F32R, tag="big_w", name="kva_w")
                nc.sync.dma_start(w[:], inn[f'kva{i}'][mb])
                p = ps.tile([128, T], F32, tag="mm0_ps", name="kva_ps")
                for kc in range(NC_D):
                    nc.tensor.matmul(p[:], w[:, kc, :], rn[:, kc, :],
                                     start=(kc == 0), stop=(kc == NC_D - 1))
                if mb < 4:
                    nc.vector.tensor_copy(kvraw[:, mb, :], p[:])
                else:
                    _rope(tc, scr, kpe[:], p[0:64, :], 64, ropec, ropes)
            _rms_norm(tc, kvraw, 4, KV_LORA, [kvn], scr, ps, ones_r, epst)

        # collective: gather latents across the 4 cores of this batch
        for lc in range(4):
            nc.sync.dma_start(ag_in[i][lc * 128:(lc + 1) * 128, :], kvn[:, lc, :])
        nc.sync.dma_start(ag_in[i][512:576, :], kpe[:])
        nc.gpsimd.collective_compute(
            "AllGather", ALU.bypass, replica_groups=groups,
            ins=[ag_in[i].opt()], outs=[ag_out[i].opt()])
        for kt in range(NKT):
            rank, half = kt // 2, kt % 2
            src = bass.AP(
                tensor=ag_out[i].tensor,
                offset=ag_out[i].offset + rank * 576 * T + half * 128,
                ap=[[T, 128], [128 * T, 4], [1, 128]])
            nc.sync.dma_start(kvk[:, :, kt * 128:(kt + 1) * 128], src)
            srcp = bass.AP(
                tensor=ag_out[i].tensor,
                offset=ag_out[i].offset + (rank * 576 + 512) * T + half * 128,
                ap=[[T, 64], [1, 128]])
            nc.sync.dma_start(kpek[0:64, kt * 128:(kt + 1) * 128], srcp)
            nc.sync.dma_start(kpek[64:128, kt * 128:(kt + 1) * 128], srcp)

        # attention core, 4 head-groups of 4 heads
        for hg in range(4):
            with (
                tc.tile_pool(name=f"a{i}g{hg}_kv", bufs=1) as kvp,
                tc.tile_pool(name=f"a{i}g{hg}_ps", bufs=2, space="PSUM") as aps,
                tc.tile_pool(name=f"a{i}g{hg}_sc", bufs=3) as asc,
            ):
                kcache = kvp.tile([128, 4, 4, T], F32R, name="kcache")
                vcache = kvp.tile([128, 2, NKT, 256], F32R, name="vcache")
                for h4 in range(4):
                    h = hg * 4 + h4
                    w = asc.tile([128, 4, 128], F32R, tag="embq_w",
                                 name="embq_w")
                    nc.sync.dma_start(w[:], inn[f'embq{i}'][h])
                    for ktp in range(4):
                        p = aps.tile([128, T], F32, tag="kv_ps", name="kb_ps")
                        for lc in range(4):
                            nc.tensor.matmul(
                                p[:], w[:, lc, :],
                                kvk[:, lc, ktp * 256:(ktp + 1) * 256],
                                start=(lc == 0), stop=(lc == 3))
                        nc.vector.tensor_copy(kcache[:, h4, ktp, :], p[:])
                for hp2 in range(2):
                    hp = hg * 2 + hp2
                    w = asc.tile([128, 4, 256], F32R, tag="unemb_w",
                                 name="unemb_w")
                    nc.sync.dma_start(w[:], inn[f'unemb{i}'][hp])
                    for kt in range(NKT):
                        p = aps.tile([128, 256], F32, tag="kv_ps", name="vb_ps")
                        for lc in range(4):
                            nc.tensor.matmul(
                                p[:], kvk[:, lc, kt * 128:(kt + 1) * 128],
                                w[:, lc, :], start=(lc == 0), stop=(lc == 3))
                        nc.vector.tensor_copy(vcache[:, hp2, kt, :], p[:])
                for h4 in range(4):
                    h = hg * 4 + h4
                    hp2, po = h4 // 2, (h4 % 2) * 128
                    avp = aps.tile([128, T], F32, tag="av_ps", bufs=1,
                                   name="av_ps")
                    denp = aps.tile([128, T], F32, tag="den_ps", bufs=1,
                                    name="den_ps")
                    for kt in range(NKT):
                        sp = aps.tile([128, T], F32, tag="s_ps", name="s_ps")
                        nc.tensor.matmul(
                            sp[:],
                            kcache[:, h4, kt // 2,
                                   (kt % 2) * 128:(kt % 2) * 128 + 128],
                            qnope[:, h, :], start=True, stop=False)
                        po2 = (h % 2) * 64
                        nc.tensor.matmul(
                            sp[:],
                            kpek[po2:po2 + 64, kt * 128:(kt + 1) * 128],
                            qpe[po2:po2 + 64, h // 2, :],
                            start=False, stop=True)
                        sm = asc.tile([128, T], F32, tag="s_m", name="s_m")
                        nc.vector.tensor_tensor(sm[:], sp[:], masks[:, kt, :],
                                                ALU.add)
                        pt = asc.tile([128, T], F32R, tag="p_t", name="p_t")
                        nc.scalar.activation(pt[:], sm[:], AF.Exp)
                        nc.tensor.matmul(avp[:],
                                         vcache[:, hp2, kt, po:po + 128],
                                         pt[:], start=(kt == 0),
                                         stop=(kt == NKT - 1))
                        nc.tensor.matmul(denp[:], ones_r[:], pt[:],
                                         start=(kt == 0), stop=(kt == NKT - 1))
                    rec = asc.tile([128, T], F32, tag="rec", name="rec")
                    nc.vector.reciprocal(rec[:], denp[:])
                    nc.vector.tensor_tensor(attn[:, h, :], avp[:], rec[:],
                                            ALU.mult)

        # o-projection + residual
        for mb in range(NC_D):
            w = wp.tile([128, NC_D, 128], F32R, tag="big_w", name="ow_w")
            nc.sync.dma_start(w[:], inn[f'ow{i}'][mb])
            p = ps.tile([128, T], F32, tag="mm0_ps", name="ow_ps")
            for kc in range(NC_D):
                nc.tensor.matmul(p[:], w[:, kc, :], attn[:, kc, :],
                                 start=(kc == 0), stop=(kc == NC_D - 1))
            nc.vector.tensor_tensor(h_t[:, mb, :], h_t[:, mb, :], p[:], ALU.add)


def _emit_ffn(tc, i, inn, dbg, h_t, short_t, ones_r, epst, postln0, ebias):
    nc = tc.nc
    with (
        tc.tile_pool(name=f"f{i}_act", bufs=1) as fact,
        tc.tile_pool(name=f"f{i}_scr", bufs=2) as fscr,
        tc.tile_pool(name=f"f{i}_w", bufs=3) as fwp,
        tc.tile_pool(name=f"f{i}_ps", bufs=2, space="PSUM") as fps,
    ):
        hn16 = fact.tile([128, NC_D, T], F16, name="hn16")
        if i == 0:
            hn32 = fact.tile([128, NC_D, T], F32, name="hn32")
            _rms_norm(tc, h_t, NC_D, D, [hn16, hn32], fscr, fps, ones_r, epst)
            if DEBUG:
                for c in range(NC_D):
                    nc.sync.dma_start(dbg["dbg_hn0"][:, c * T:(c + 1) * T],
                                      hn32[:, c, :])
            _emit_moe(tc, inn, dbg, hn16, hn32, short_t, postln0, ebias,
                      fact, fscr, fwp, fps)
            if DEBUG:
                for c in range(NC_D):
                    nc.sync.dma_start(dbg["dbg_short"][:, c * T:(c + 1) * T],
                                      short_t[:, c, :])
        else:
            _rms_norm(tc, h_t, NC_D, D, [hn16], fscr, fps, ones_r, epst)

        acts = fact.tile([128, 64, T], F16, name="ffn_acts")
        for mb in range(64):
            wg = fwp.tile([128, NC_D, 128], F16, tag="ffn_w", name="gate_w")
            nc.sync.dma_start(wg[:], inn[f'gate{i}'][mb])
            gp = fps.tile([128, T], F32, tag="mm0_ps", name="g_ps")
            for kc in range(NC_D):
                nc.tensor.matmul(gp[:], wg[:, kc, :], hn16[:, kc, :],
                                 start=(kc == 0), stop=(kc == NC_D - 1))
            wu = fwp.tile([128, NC_D, 128], F16, tag="ffn_w", name="up_w")
            nc.sync.dma_start(wu[:], inn[f'up{i}'][mb])
            upp = fps.tile([128, T], F32, tag="u_ps", name="u_ps")
            for kc in range(NC_D):
                nc.tensor.matmul(upp[:], wu[:, kc, :], hn16[:, kc, :],
                                 start=(kc == 0), stop=(kc == NC_D - 1))
            sg = fscr.tile([128, T], F16, tag="silu", name="silu_g")
            nc.scalar.activation(sg[:], gp[:], AF.Silu)
            nc.vector.tensor_tensor(acts[:, mb, :], sg[:], upp[:], ALU.mult)
        for mb2 in range(NC_D):
            wd = fwp.tile([128, 64, 128], F16, tag="ffn_wd", bufs=2,
                          name="down_w")
            nc.sync.dma_start(wd[:], inn[f'down{i}'][mb2])
            dp = fps.tile([128, T], F32, tag="d_ps", name="d_ps")
            for kc in range(64):
                nc.tensor.matmul(dp[:], wd[:, kc, :], acts[:, kc, :],
                                 start=(kc == 0), stop=(kc == 63))
            nc.vector.tensor_tensor(h_t[:, mb2, :], h_t[:, mb2, :], dp[:],
                                    ALU.add)


def _emit_moe(tc, inn, dbg, hn16, hn32, short_t, postln0, ebias,
              fact, fscr, fwp, fps):
    nc = tc.nc
    with tc.tile_pool(name="moe_dram", bufs=1, space="DRAM") as mdram:
        router_sb = fact.tile([128, NC_D, NE + NZ], F32, name="router_sb")
        nc.sync.dma_start(router_sb[:], inn['router'][:])
        ce_dram = mdram.tile([2, 128, NE + NZ], F32, name="ce_dram")
        idw_dram = mdram.tile([2, 128, 1], F32, name="idw_dram")
        for tt in range(2):
            rp = fps.tile([128, NE + NZ], F32, tag="r_ps", bufs=1, name="r_ps")
            for kc in range(NC_D):
                nc.tensor.matmul(rp[:], hn32[:, kc, tt * 128:(tt + 1) * 128],
                                 router_sb[:, kc, :],
                                 start=(kc == 0), stop=(kc == NC_D - 1))
            mx = fscr.tile([128, 1], F32, tag="r_mx", name="r_mx")
            nc.vector.reduce_max(mx[:], rp[:], axis=AX.X)
            nmx = fscr.tile([128, 1], F32, tag="r_nmx", name="r_nmx")
            nc.vector.tensor_scalar(nmx[:], mx[:], -1.0, None, ALU.mult)
            ex = fscr.tile([128, NE + NZ], F32, tag="r_ex", name="r_ex")
            nc.scalar.activation(ex[:], rp[:], AF.Exp, bias=nmx[:])
            sm = fscr.tile([128, 1], F32, tag="r_sm", name="r_sm")
            nc.vector.reduce_sum(sm[:], ex[:], axis=AX.X)
            rc = fscr.tile([128, 1], F32, tag="r_rc", name="r_rc")
            nc.vector.reciprocal(rc[:], sm[:])
            sc = fscr.tile([128, NE + NZ], F32, tag="r_sc", name="r_sc")
            nc.vector.tensor_scalar(sc[:], ex[:], rc[:], None, ALU.mult)
            bi = fscr.tile([128, NE + NZ], F32, tag="r_bi", name="r_bi")
            nc.vector.tensor_tensor(bi[:], sc[:], ebias[:], ALU.add)
            m1 = fscr.tile([128, 1], F32, tag="r_m1", name="r_m1")
            nc.vector.reduce_max(m1[:], bi[:], axis=AX.X)
            eq1 = fscr.tile([128, NE + NZ], F32, tag="r_eq1", name="r_eq1")
            nc.vector.tensor_scalar(eq1[:], bi[:], m1[:], None, ALU.is_equal)
            mk = fscr.tile([128, NE + NZ], F32, tag="r_mk", name="r_mk")
            nc.vector.scalar_tensor_tensor(mk[:], eq1[:], -1e30, bi[:],
                                           ALU.mult, ALU.add)
            m2 = fscr.tile([128, 1], F32, tag="r_m2", name="r_m2")
            nc.vector.reduce_max(m2[:], mk[:], axis=AX.X)
            sel = fscr.tile([128, NE + NZ], F32, tag="r_sel", name="r_sel")
            nc.vector.tensor_scalar(sel[:], bi[:], m2[:], None, ALU.is_ge)
            ce = fscr.tile([128, NE + NZ], F32, tag="r_ce", name="r_ce")
            nc.vector.scalar_tensor_tensor(ce[:], sc[:], RSF, sel[:],
                                           ALU.mult, ALU.mult)
            nc.sync.dma_start(ce_dram[tt], ce[:])
            if DEBUG:
                nc.sync.dma_start(dbg["dbg_ce"][tt], ce[:])
            idw = fscr.tile([128, 1], F32, tag="r_idw", name="r_idw")
            nc.vector.reduce_sum(idw[:], ce[:, NE:NE + NZ], axis=AX.X)
            nc.sync.dma_start(idw_dram[tt], idw[:])

        ceb = fact.tile([128, NE, T], F32, name="ceb")
        for e in range(NE):
            src = ce_dram[:, :, e]
            bsrc = bass.AP(tensor=src.tensor, offset=src.offset,
                           ap=[[0, 128]] + [list(d) for d in src.ap])
            nc.sync.dma_start(ceb[:, e, :], bsrc)
        idwb = fact.tile([128, T], F32, name="idwb")
        src = idw_dram[:, :, 0]
        bsrc = bass.AP(tensor=src.tensor, offset=src.offset,
                       ap=[[0, 128]] + [list(d) for d in src.ap])
        nc.sync.dma_start(idwb[:], bsrc)

        for c in range(NC_D):
            t1 = fscr.tile([128, T], F32, tag="id_t1", name="id_t1")
            nc.vector.tensor_scalar(t1[:], hn32[:, c, :], postln0[:, c:c + 1],
                                    None, ALU.mult)
            nc.vector.tensor_tensor(short_t[:, c, :], t1[:], idwb[:], ALU.mult)

        eacts = fact.tile([128, NE, 8, T], F16, name="eacts")
        for e in range(NE):
            for mb in range(8):
                wg = fwp.tile([128, NC_D, 128], F16, tag="ffn_w", name="eg_w")
                nc.sync.dma_start(wg[:], inn['egate'][e, mb])
                gp = fps.tile([128, T], F32, tag="mm0_ps", name="eg_ps")
                for kc in range(NC_D):
                    nc.tensor.matmul(gp[:], wg[:, kc, :], hn16[:, kc, :],
                                     start=(kc == 0), stop=(kc == NC_D - 1))
                wu = fwp.tile([128, NC_D, 128], F16, tag="ffn_w", name="eu_w")
                nc.sync.dma_start(wu[:], inn['eup'][e, mb])
                upp = fps.tile([128, T], F32, tag="u_ps", name="eu_ps")
                for kc in range(NC_D):
                    nc.tensor.matmul(upp[:], wu[:, kc, :], hn16[:, kc, :],
                                     start=(kc == 0), stop=(kc == NC_D - 1))
                sg = fscr.tile([128, T], F16, tag="silu", name="e_silu")
                nc.scalar.activation(sg[:], gp[:], AF.Silu)
                su = fscr.tile([128, T], F16, tag="su", name="e_su")
                nc.vector.tensor_tensor(su[:], sg[:], upp[:], ALU.mult)
                nc.vector.tensor_tensor(eacts[:, e, mb, :], su[:],
                                        ceb[:, e, :], ALU.mult)
        for mb2 in range(NC_D):
            dp = fps.tile([128, T], F32, tag="d_ps", name="ed_ps")
            first = True
            for e in range(NE):
                wd = fwp.tile([128, 8, 128], F16, tag="e_wd", bufs=2,
                              name="ed_w")
                nc.sync.dma_start(wd[:], inn['edown'][e, mb2])
                for kc in range(8):
                    nc.tensor.matmul(dp[:], wd[:, kc, :], eacts[:, e, kc, :],
                                     start=first,
                                     stop=(e == NE - 1 and kc == 7))
                    first = False
            nc.vector.tensor_tensor(short_t[:, mb2, :], short_t[:, mb2, :],
                                    dp[:], ALU.add)


_PROGRAM = [None]


def kernel(**inputs):
    if _PROGRAM[0] is None:
        _PROGRAM[0] = _build_program()
    nc = _PROGRAM[0]
    in_maps = _prep_inputs(inputs)
    trace = bool(int(os.environ.get("KERNEL_TRACE", "0")))
    res = run_bass_kernel_spmd(nc, in_maps, list(range(N_CORES)), trace=trace)
    kernel._exec_time_ns = res.exec_time_ns
    out = np.empty((B, L, D), np.float32)
    for c in range(N_CORES):
        b, blk = c // 4, c % 4
        oc = res.results[c]["out"].reshape(128, NC_D, T)
        out[b, blk * T:(blk + 1) * T, :] = oc.transpose(2, 1, 0).reshape(T, D)
    if DEBUG:
        kernel._debug = res.results
    return out
